# revision 2
# baseline (speedup 1.0000x reference)
"""Distributed embedding-lookup kernel for 8 TRN2 NeuronCores (Bass/Tile), v4.

Computes, for full inputs:
    word_sum = sum(word_matrix[context_ids], axis=1)        # [B, D]
    inputs   = paragraph_matrix[doc_ids] + word_sum         # [B, D]
    out_cols = outputs[:, sample_ids]                       # [D, B, S]
    logits   = einsum("bd,dbs->bs", inputs, out_cols)       # [B, S]

Strategy (SPMD, one NEFF on 8 cores; batch-sharded, no collectives):
  Everything is bf16 and goes through SWDGE dma_gather with QUAD-row
  elements: elem = 4 consecutive table rows = 1KB, so idx = id//4 <=
  24999 fits int16 with a SINGLE window over the whole 100k-row table,
  and descriptor cost stays 2 descs/idx (PACKET_BYTES=16KB).  That
  allows gathering directly in compute order with no staging:
    - doc quads in b order -> [b%128, b//128, 4, D]
    - ctx quads in (b, e) slot order, 8 segments of 2304
    - outputs^T (host-transposed) quads in (j, b//128, b%128) order,
      12 segments (one per (sample j, b-quarter))
  The 1-of-4 row select folds into a single DVE multiply by an uploaded
  one-hot [slot, 4] mask (broadcast over D).  The sum over the 8 ctx
  entries is a static block-diagonal PE matmul (S[r, m] = r//8 == m),
  which also sums the 4 masked quad rows; a strided DVE reduce folds
  the remaining 4-way sum.  Phase B pairs each gathered T quad with
  inputs[b] via a pure broadcast AP (no gather), multiplies by the
  one-hot mask, and reduces (q, d) on DVE -> logits.
"""

import sys
import types

import numpy as np

try:
    from ml_dtypes import bfloat16 as np_bf16
except ImportError:  # pragma: no cover
    np_bf16 = None

# ---------------------------------------------------------------------------
B = 16384
D = 128
CTX = 8
S = 6
V = 100000
N_CORES = 8
BL = B // N_CORES              # 2048 batch rows per core
NQ = V // 4                    # 25000 quad rows

NCTX = BL * CTX                # 16384 ctx slots (b-major, 8 per b)
CSEG = 16                      # ctx gather segments (one 128-b group each)
CSN = NCTX // CSEG             # 1024 idxs per ctx call
CTILE = CSN // 128             # 8 slot-tiles per ctx segment

NSMP = BL * S                  # 12288 sample slots
TSEG = 12                      # T gather segments
TSN = NSMP // TSEG             # 1024 idxs per T call (8 b-quarters x 128)
TTILE = TSN // 128             # 8

BQ = BL // 128                 # 16 b-quarters

IDXC = (BL + NCTX + NSMP) // 16   # 1920 wrap16 idx columns

_nc_cache = None


def _install_ntff_hook():
    """antenv.axon_hooks is absent from this image; inject it so
    run_bass_kernel_spmd(trace=True) can capture NTFF profiles."""
    if "antenv.axon_hooks" in sys.modules:
        return
    mod = types.ModuleType("antenv.axon_hooks")
    mod._hook = None
    mod.set_axon_ntff_profile_hook = lambda h: setattr(mod, "_hook", h)
    mod.get_axon_ntff_profile_hook = lambda: mod._hook
    sys.modules["antenv.axon_hooks"] = mod
    try:
        import antenv
        antenv.axon_hooks = mod
        from trn_agent_boot.trn_boot import _ntff_profile_via_ctypes
        mod.set_axon_ntff_profile_hook(
            _ntff_profile_via_ctypes("/opt/axon/libaxon_pjrt.so"))
    except Exception:
        pass


def _patch_swdge_lane_assignment():
    """Pin queue-tagged SWDGE ops (dma_gather) to sem lane == queue_num and
    round-robin untagged SWDGE DMAs over lanes 4..7 (the runtime locks each
    sem lane to the first SWDGE queue that increments it)."""
    import concourse.tile_sem_assignment as tsa
    import concourse.mybir as mybir
    from concourse import bass_isa

    if getattr(tsa.TileClockTick, "_lane_patch", False):
        return
    orig = tsa.TileClockTick._assign_tick

    def _assign_tick(self, inst):
        if (
            isinstance(inst, tsa.DMAInst)
            and not isinstance(inst, bass_isa.UserSyncedRemoteDMADescs)
            and inst.engine == mybir.EngineType.Pool
        ):
            qn = getattr(inst, "queue_num", None)
            if isinstance(qn, int) and 0 <= qn <= 3:
                lane = qn
            else:
                lane = 4 + self.next_sw_dma_idx % 4
                self.next_sw_dma_idx += 1
            proc = tsa.PROC_NAME_TO_IDX[f"DMASW{lane}"]
            inst.bass_scheduled_tick = self.global_clock.advance(proc)
            inst.bass_scheduled_proc = proc
            inst.bass_scheduled_scope = self.scope_name
            self._proc_insts[self.root_scope_name][proc].append(inst)
            eng_proc = tsa.ENGINE_TO_IDX[inst.engine]
            if getattr(inst, "gen_mode", 0) == 1 and proc != eng_proc:
                eng_tick = self.global_clock.advance(eng_proc)
                self.tc.prep_eng_ticks[inst.name] = (eng_proc, eng_tick)
                self._prep_eng_names[self.root_scope_name].append(inst.name)
            return
        return orig(self, inst)

    tsa.TileClockTick._assign_tick = _assign_tick
    tsa.TileClockTick._lane_patch = True


def _build_nc():
    import concourse.bacc as bacc
    import concourse.mybir as mybir
    import concourse.tile as tile

    _patch_swdge_lane_assignment()

    f32 = mybir.dt.float32
    bf16 = mybir.dt.bfloat16
    i16 = mybir.dt.int16

    nc = bacc.Bacc("TRN2", target_bir_lowering=False, debug=False,
                   num_devices=N_CORES, num_swdge_queues=4)

    wt = nc.dram_tensor("wt", [V, D], bf16, kind="ExternalInput")
    tt = nc.dram_tensor("tt", [V, D], bf16, kind="ExternalInput")
    pt = nc.dram_tensor("pt", [V, D], bf16, kind="ExternalInput")
    idx_d = nc.dram_tensor("idx", [128, IDXC], i16, kind="ExternalInput")
    # one-hot quad masks, expanded over d: [slot, 4, D] bf16
    qmd_d = nc.dram_tensor("qmd", [128, BQ * 512], bf16,
                           kind="ExternalInput")
    qmc_d = nc.dram_tensor("qmc", [128, (NCTX // 128) * 512], bf16,
                           kind="ExternalInput")
    qmt_d = nc.dram_tensor("qmt", [128, (NSMP // 128) * 512], bf16,
                           kind="ExternalInput")
    # 8 shifted one-hot S matrices + identity
    smat_d = nc.dram_tensor("smat", [128, 1152], bf16,
                            kind="ExternalInput")
    vals_d = nc.dram_tensor("vals", [128, S * BQ], f32, kind="ExternalOutput")

    wq = wt[:].rearrange("(n four) d -> n (four d)", four=4)
    tq = tt[:].rearrange("(n four) d -> n (four d)", four=4)
    pq = pt[:].rearrange("(n four) d -> n (four d)", four=4)

    with tile.TileContext(nc) as tc:
        with (
            tc.tile_pool(name="const", bufs=1) as cpool,
            tc.tile_pool(name="acc", bufs=1) as apool,
            tc.tile_pool(name="doc", bufs=1) as dpool,
            tc.tile_pool(name="ctx", bufs=2) as xpool,
            tc.tile_pool(name="cmsk", bufs=2) as cmpool,
            tc.tile_pool(name="tg", bufs=3) as tpool,
            tc.tile_pool(name="tmsk", bufs=2) as tmpool,
            tc.tile_pool(name="lg", bufs=1) as lpool,
            tc.tile_pool(name="psum", bufs=4, space="PSUM") as pspool,
        ):
            idx_sb = cpool.tile([128, IDXC], i16)
            nc.sync.dma_start(idx_sb[:], idx_d[:])
            smat = cpool.tile([128, 1152], bf16)
            nc.sync.dma_start(smat[:], smat_d[:])
            ident = smat[:, 1024:1152]

            icol = 0

            def idxs(n):
                nonlocal icol
                ap = idx_sb[:, icol:icol + n // 16]
                icol += n // 16
                return ap

            doc_idx = idxs(BL)
            ctx_idx = [idxs(CSN) for _ in range(CSEG)]
            t_idx = [idxs(TSN) for _ in range(TSEG)]

            accf = apool.tile([128, BQ * D], f32)      # [b%128, b//128, d]
            accx = apool.tile([128, BQ * 512], bf16)   # [b%128, bq, q, d]
            lg = lpool.tile([128, S * BQ], f32)        # [b%128, (j, b//128)]

            qn = 0

            # ---- doc quads: slot b -> [b%128, b//128, 4, 128] -----------
            docq = dpool.tile([128, BQ * 512], bf16)
            nc.gpsimd.dma_gather(
                out_ap=docq[:].rearrange("p (c e) -> p c e", e=512),
                in_ap=pq,
                idxs_ap=doc_idx,
                num_idxs=BL, num_idxs_reg=BL, elem_size=512,
                queue_num=qn % 4, single_packet=False,
            )
            qn += 1
            qmdf = dpool.tile([128, BQ * 512], bf16)
            nc.scalar.dma_start(qmdf[:], qmd_d[:])
            nc.vector.tensor_mul(docq[:], docq[:], qmdf[:])

            # ---- ctx + T gathers, interleaved desc-gen ------------------
            # seg s covers slots [1024 s, 1024 (s+1)) = b-group s
            def emit_tseg(sT):
                nonlocal qn
                tg = tpool.tile([128, TTILE * 512], bf16)
                nc.gpsimd.dma_gather(
                    out_ap=tg[:].rearrange("p (c e) -> p c e", e=512),
                    in_ap=tq,
                    idxs_ap=t_idx[sT],
                    num_idxs=TSN, num_idxs_reg=TSN, elem_size=512,
                    queue_num=qn % 4, single_packet=False,
                )
                qn += 1
                amx = tmpool.tile([128, TTILE * 512], bf16)
                nc.scalar.dma_start(
                    amx[:], qmt_d[:, sT * TTILE * 512:(sT + 1) * TTILE * 512])
                bq0 = (sT % 2) * TTILE
                nc.vector.tensor_mul(
                    amx[:], amx[:],
                    accx[:, bq0 * 512:(bq0 + TTILE) * 512])
                nc.vector.tensor_mul(tg[:], tg[:], amx[:])
                nc.vector.reduce_sum(
                    lg[:, sT * TTILE:(sT + 1) * TTILE]
                    .rearrange("p (c one) -> p c one", one=1),
                    tg[:].rearrange("p (c e) -> p c e", e=512),
                    axis=mybir.AxisListType.X,
                )

            for s in range(CSEG):
                cx = xpool.tile([128, CTILE * 512], bf16)
                nc.gpsimd.dma_gather(
                    out_ap=cx[:].rearrange("p (c e) -> p c e", e=512),
                    in_ap=wq,
                    idxs_ap=ctx_idx[s],
                    num_idxs=CSN, num_idxs_reg=CSN, elem_size=512,
                    queue_num=qn % 4, single_packet=False,
                )
                qn += 1
                cmf = cmpool.tile([128, CTILE * 512], bf16)
                nc.scalar.dma_start(
                    cmf[:], qmc_d[:, s * CTILE * 512:(s + 1) * CTILE * 512])
                nc.vector.tensor_mul(cx[:], cx[:], cmf[:])
                # segmented sum for b-group s: 8 shifted one-hot S matrices
                # + identity for the doc quads, accumulated in one psum
                ps = pspool.tile([128, 512], f32)
                for t in range(8):
                    nc.tensor.matmul(ps[:],
                                     lhsT=smat[:, t * 128:(t + 1) * 128],
                                     rhs=cx[:, t * 512:(t + 1) * 512],
                                     start=(t == 0), stop=False)
                nc.tensor.matmul(ps[:], lhsT=ident,
                                 rhs=docq[:, s * 512:(s + 1) * 512],
                                 start=False, stop=True)
                nc.vector.reduce_sum(
                    accf[:, s * D:(s + 1) * D]
                    .rearrange("p (one d) -> p one d", one=1),
                    ps[:].rearrange("p (q d) -> p d q", q=4, d=D),
                    axis=mybir.AxisListType.X,
                )
                for q in range(4):
                    nc.vector.tensor_copy(
                        accx[:, s * 512 + q * D:s * 512 + (q + 1) * D],
                        accf[:, s * D:(s + 1) * D])
                # phase B segs become eligible as their b-groups complete:
                # even T segs need groups 0..7, odd need 8..15
                if s == 7:
                    for sT in range(0, TSEG, 2):
                        emit_tseg(sT)
                elif s == 15:
                    for sT in range(1, TSEG, 2):
                        emit_tseg(sT)

            nc.sync.dma_start(vals_d[:], lg[:])

    nc.compile()
    return nc


def _get_nc():
    global _nc_cache
    if _nc_cache is None:
        _nc_cache = _build_nc()
    return _nc_cache


def _wrap16(flat):
    """[n] int array (n % 16 == 0) -> [128, n//16] int16 laid out as the
    gpsimd ucode reads it: idx j at (partition j%16, col j//16),
    replicated across the eight 16-partition groups."""
    m = np.asarray(flat, dtype=np.int16).reshape(-1, 16).T  # [16, n//16]
    return np.tile(m, (8, 1))


def _to_bf16(a):
    a = np.asarray(a, dtype=np.float32)
    if np_bf16 is not None:
        return a.astype(np_bf16)
    u = a.view(np.uint32)
    u = ((u + 0x7FFF + ((u >> 16) & 1)) >> 16).astype(np.uint16)
    return u


def _onehot_mask(sel, ntiles):
    """sel: [n] in {0..3}, slot i at gather position (i%128, i//128).
    Returns [128, ntiles*512] bf16 one-hot expanded over the 128 dims:
    m[p, ((c*4+q)*128):+128] = (sel[c*128+p]==q)."""
    n = sel.shape[0]
    assert n == ntiles * 128
    m = np.zeros((128, ntiles, 4), dtype=np.float32)
    pos = np.arange(n)
    m[pos % 128, pos // 128, sel] = 1.0
    m = np.repeat(m.reshape(128, ntiles * 4), 128, axis=1)
    return _to_bf16(m)


def _prepare_core(k, doc_ids, context_ids, sample_ids):
    """Host-side index prep for core k (pure index arithmetic)."""
    bsl = slice(k * BL, (k + 1) * BL)
    doc = np.asarray(doc_ids[bsl], dtype=np.int64)          # [BL]
    ctx = np.asarray(context_ids[bsl], dtype=np.int64)      # [BL, CTX]
    smp = np.asarray(sample_ids[bsl], dtype=np.int64)       # [BL, S]

    segs = [_wrap16(doc // 4)]
    ctxf = ctx.reshape(-1)
    for s in range(CSEG):
        segs.append(_wrap16(ctxf[s * CSN:(s + 1) * CSN] // 4))
    # T slots: slot (b, j) at gather index i = (j*16 + b//128)*128 + b%128
    b = np.repeat(np.arange(BL), S).reshape(BL, S)
    i_of = (smp * 0 + (np.arange(S)[None, :] * BQ + b // 128) * 128
            + b % 128)
    tflat = np.empty(NSMP, dtype=np.int64)
    tflat[i_of.reshape(-1)] = smp.reshape(-1)
    for s in range(TSEG):
        segs.append(_wrap16(tflat[s * TSN:(s + 1) * TSN] // 4))
    idx_all = np.concatenate(segs, axis=1)
    assert idx_all.shape == (128, IDXC), idx_all.shape

    qmd = _onehot_mask(doc % 4, BQ)
    qmc = _onehot_mask(ctxf % 4, NCTX // 128)
    qmt = _onehot_mask(tflat % 4, NSMP // 128)
    return idx_all, qmd, qmc, qmt


def _run(doc_ids, context_ids, sample_ids, paragraph_matrix, word_matrix,
         outputs, trace=False):
    _install_ntff_hook()
    from concourse.bass_utils import run_bass_kernel_spmd

    nc = _get_nc()

    wt_b = _to_bf16(np.asarray(word_matrix, dtype=np.float32))
    tt_b = _to_bf16(np.ascontiguousarray(
        np.asarray(outputs, dtype=np.float32).T))
    pt_b = _to_bf16(np.asarray(paragraph_matrix, dtype=np.float32))
    smat = np.zeros((128, 9, 128), dtype=np.float32)
    r = np.arange(128)
    for t in range(8):
        smat[r, t, r // 8 + 16 * t] = 1.0
    smat[r, 8, r] = 1.0
    smat_b = _to_bf16(smat.reshape(128, 1152))

    in_maps = []
    for k in range(N_CORES):
        idx_all, qmd, qmc, qmt = _prepare_core(k, doc_ids, context_ids,
                                               sample_ids)
        in_maps.append({
            "wt": wt_b,
            "tt": tt_b,
            "pt": pt_b,
            "idx": idx_all,
            "qmd": qmd,
            "qmc": qmc,
            "qmt": qmt,
            "smat": smat_b,
        })

    res = run_bass_kernel_spmd(nc, in_maps, core_ids=list(range(N_CORES)),
                               trace=trace)

    logits = np.zeros((B, S), dtype=np.float32)
    bl = np.arange(BL)
    for k in range(N_CORES):
        vals = np.asarray(res.results[k]["vals"],
                          dtype=np.float32).reshape(128, S, BQ)
        logits[k * BL:(k + 1) * BL] = vals[bl[:, None] % 128,
                                           np.arange(S)[None, :],
                                           bl[:, None] // 128]
    return logits, res


def kernel(doc_ids, context_ids, sample_ids, paragraph_matrix, word_matrix,
           outputs):
    logits, _ = _run(doc_ids, context_ids, sample_ids, paragraph_matrix,
                     word_matrix, outputs, trace=False)
    return logits


def kernel_traced(doc_ids, context_ids, sample_ids, paragraph_matrix,
                  word_matrix, outputs):
    """Same as kernel() but captures an NTFF profile; returns
    (logits, exec_time_ns)."""
    logits, res = _run(doc_ids, context_ids, sample_ids, paragraph_matrix,
                       word_matrix, outputs, trace=True)
    return logits, res.exec_time_ns


# revision 3
# speedup vs baseline: 1.1975x; 1.1975x over previous
"""Distributed embedding-lookup kernel for 8 TRN2 NeuronCores (Bass/Tile), v4.

Computes, for full inputs:
    word_sum = sum(word_matrix[context_ids], axis=1)        # [B, D]
    inputs   = paragraph_matrix[doc_ids] + word_sum         # [B, D]
    out_cols = outputs[:, sample_ids]                       # [D, B, S]
    logits   = einsum("bd,dbs->bs", inputs, out_cols)       # [B, S]

Strategy (SPMD, one NEFF on 8 cores; batch-sharded, no collectives):
  Everything is bf16 and goes through SWDGE dma_gather with QUAD-row
  elements: elem = 4 consecutive table rows = 1KB, so idx = id//4 <=
  24999 fits int16 with a SINGLE window over the whole 100k-row table,
  and descriptor cost stays 2 descs/idx (PACKET_BYTES=16KB).  That
  allows gathering directly in compute order with no staging:
    - doc quads in b order -> [b%128, b//128, 4, D]
    - ctx quads in (b, e) slot order, 8 segments of 2304
    - outputs^T (host-transposed) quads in (j, b//128, b%128) order,
      12 segments (one per (sample j, b-quarter))
  The 1-of-4 row select folds into a single DVE multiply by an uploaded
  one-hot [slot, 4] mask (broadcast over D).  The sum over the 8 ctx
  entries is a static block-diagonal PE matmul (S[r, m] = r//8 == m),
  which also sums the 4 masked quad rows; a strided DVE reduce folds
  the remaining 4-way sum.  Phase B pairs each gathered T quad with
  inputs[b] via a pure broadcast AP (no gather), multiplies by the
  one-hot mask, and reduces (q, d) on DVE -> logits.
"""

import sys
import types

import numpy as np

try:
    from ml_dtypes import bfloat16 as np_bf16
except ImportError:  # pragma: no cover
    np_bf16 = None

# ---------------------------------------------------------------------------
B = 16384
D = 128
CTX = 8
S = 6
V = 100000
N_CORES = 8
BL = B // N_CORES              # 2048 batch rows per core
NQ = V // 4                    # 25000 quad rows

NCTX = BL * CTX                # 16384 ctx slots (b-major, 8 per b)
CSEG = 16                      # ctx gather segments (one 128-b group each)
CSN = NCTX // CSEG             # 1024 idxs per ctx call
CTILE = CSN // 128             # 8 slot-tiles per ctx segment

NSMP = BL * S                  # 12288 sample slots
TSEG = 24                      # T gather segments
TSN = NSMP // TSEG             # 512 idxs per T call (4 b-quarters x 128)
TTILE = TSN // 128             # 4

BQ = BL // 128                 # 16 b-quarters

IDXC = (BL + NCTX + NSMP) // 16   # 1920 wrap16 idx columns

_nc_cache = None


def _install_ntff_hook():
    """antenv.axon_hooks is absent from this image; inject it so
    run_bass_kernel_spmd(trace=True) can capture NTFF profiles."""
    if "antenv.axon_hooks" in sys.modules:
        return
    mod = types.ModuleType("antenv.axon_hooks")
    mod._hook = None
    mod.set_axon_ntff_profile_hook = lambda h: setattr(mod, "_hook", h)
    mod.get_axon_ntff_profile_hook = lambda: mod._hook
    sys.modules["antenv.axon_hooks"] = mod
    try:
        import antenv
        antenv.axon_hooks = mod
        from trn_agent_boot.trn_boot import _ntff_profile_via_ctypes
        mod.set_axon_ntff_profile_hook(
            _ntff_profile_via_ctypes("/opt/axon/libaxon_pjrt.so"))
    except Exception:
        pass


def _patch_swdge_lane_assignment():
    """Pin queue-tagged SWDGE ops (dma_gather) to sem lane == queue_num and
    round-robin untagged SWDGE DMAs over lanes 4..7 (the runtime locks each
    sem lane to the first SWDGE queue that increments it)."""
    import concourse.tile_sem_assignment as tsa
    import concourse.mybir as mybir
    from concourse import bass_isa

    if getattr(tsa.TileClockTick, "_lane_patch", False):
        return
    orig = tsa.TileClockTick._assign_tick

    def _assign_tick(self, inst):
        if (
            isinstance(inst, tsa.DMAInst)
            and not isinstance(inst, bass_isa.UserSyncedRemoteDMADescs)
            and inst.engine == mybir.EngineType.Pool
        ):
            qn = getattr(inst, "queue_num", None)
            if isinstance(qn, int) and 0 <= qn <= 3:
                lane = qn
            else:
                lane = 4 + self.next_sw_dma_idx % 4
                self.next_sw_dma_idx += 1
            proc = tsa.PROC_NAME_TO_IDX[f"DMASW{lane}"]
            inst.bass_scheduled_tick = self.global_clock.advance(proc)
            inst.bass_scheduled_proc = proc
            inst.bass_scheduled_scope = self.scope_name
            self._proc_insts[self.root_scope_name][proc].append(inst)
            eng_proc = tsa.ENGINE_TO_IDX[inst.engine]
            if getattr(inst, "gen_mode", 0) == 1 and proc != eng_proc:
                eng_tick = self.global_clock.advance(eng_proc)
                self.tc.prep_eng_ticks[inst.name] = (eng_proc, eng_tick)
                self._prep_eng_names[self.root_scope_name].append(inst.name)
            return
        return orig(self, inst)

    tsa.TileClockTick._assign_tick = _assign_tick
    tsa.TileClockTick._lane_patch = True


def _build_nc():
    import concourse.bacc as bacc
    import concourse.mybir as mybir
    import concourse.tile as tile

    _patch_swdge_lane_assignment()

    f32 = mybir.dt.float32
    bf16 = mybir.dt.bfloat16
    i16 = mybir.dt.int16

    nc = bacc.Bacc("TRN2", target_bir_lowering=False, debug=False,
                   num_devices=N_CORES, num_swdge_queues=4)

    wt = nc.dram_tensor("wt", [V, D], bf16, kind="ExternalInput")
    tt = nc.dram_tensor("tt", [V, D], bf16, kind="ExternalInput")
    pt = nc.dram_tensor("pt", [V, D], bf16, kind="ExternalInput")
    idx_d = nc.dram_tensor("idx", [128, IDXC], i16, kind="ExternalInput")
    # one-hot quad masks, expanded over d: [slot, 4, D] bf16
    qmd_d = nc.dram_tensor("qmd", [128, BQ * 512], bf16,
                           kind="ExternalInput")
    qmc_d = nc.dram_tensor("qmc", [128, (NCTX // 128) * 512], bf16,
                           kind="ExternalInput")
    qmt_d = nc.dram_tensor("qmt", [128, (NSMP // 128) * 512], bf16,
                           kind="ExternalInput")
    # 8 shifted one-hot S matrices + identity
    smat_d = nc.dram_tensor("smat", [128, 1152], bf16,
                            kind="ExternalInput")
    vals_d = nc.dram_tensor("vals", [128, S * BQ], f32, kind="ExternalOutput")

    wq = wt[:].rearrange("(n four) d -> n (four d)", four=4)
    tq = tt[:].rearrange("(n four) d -> n (four d)", four=4)
    pq = pt[:].rearrange("(n four) d -> n (four d)", four=4)

    with tile.TileContext(nc) as tc:
        with (
            tc.tile_pool(name="const", bufs=1) as cpool,
            tc.tile_pool(name="acc", bufs=1) as apool,
            tc.tile_pool(name="doc", bufs=1) as dpool,
            tc.tile_pool(name="ctx", bufs=3) as xpool,
            tc.tile_pool(name="cmsk", bufs=2) as cmpool,
            tc.tile_pool(name="tg", bufs=6) as tpool,
            tc.tile_pool(name="tmsk", bufs=3) as tmpool,
            tc.tile_pool(name="lg", bufs=1) as lpool,
            tc.tile_pool(name="psum", bufs=4, space="PSUM") as pspool,
        ):
            idx_sb = cpool.tile([128, IDXC], i16)
            nc.sync.dma_start(idx_sb[:], idx_d[:])
            smat = cpool.tile([128, 1152], bf16)
            nc.sync.dma_start(smat[:], smat_d[:])
            ident = smat[:, 1024:1152]

            icol = 0

            def idxs(n):
                nonlocal icol
                ap = idx_sb[:, icol:icol + n // 16]
                icol += n // 16
                return ap

            doc_idx = idxs(BL)
            ctx_idx = [idxs(CSN) for _ in range(CSEG)]
            t_idx = [idxs(TSN) for _ in range(TSEG)]

            accf = apool.tile([128, BQ * D], f32)      # [b%128, b//128, d]
            accx = apool.tile([128, BQ * 512], bf16)   # [b%128, bq, q, d]
            lg = lpool.tile([128, S * BQ], f32)        # [b%128, (j, b//128)]

            qn = 0

            # ---- doc quads: slot b -> [b%128, b//128, 4, 128] -----------
            docq = dpool.tile([128, BQ * 512], bf16)
            nc.gpsimd.dma_gather(
                out_ap=docq[:].rearrange("p (c e) -> p c e", e=512),
                in_ap=pq,
                idxs_ap=doc_idx,
                num_idxs=BL, num_idxs_reg=BL, elem_size=512,
                queue_num=qn % 4, single_packet=False,
            )
            qn += 1
            qmdf = dpool.tile([128, BQ * 512], bf16)
            nc.scalar.dma_start(qmdf[:], qmd_d[:])
            nc.vector.tensor_mul(docq[:], docq[:], qmdf[:])

            # ---- ctx + T gathers, interleaved desc-gen ------------------
            # seg s covers slots [1024 s, 1024 (s+1)) = b-group s
            def emit_tseg(sT):
                nonlocal qn
                tg = tpool.tile([128, TTILE * 512], bf16)
                nc.gpsimd.dma_gather(
                    out_ap=tg[:].rearrange("p (c e) -> p c e", e=512),
                    in_ap=tq,
                    idxs_ap=t_idx[sT],
                    num_idxs=TSN, num_idxs_reg=TSN, elem_size=512,
                    queue_num=qn % 4, single_packet=False,
                )
                qn += 1
                amx = tmpool.tile([128, TTILE * 512], bf16)
                nc.scalar.dma_start(
                    amx[:], qmt_d[:, sT * TTILE * 512:(sT + 1) * TTILE * 512])
                bq0 = (sT % 4) * TTILE
                nc.vector.tensor_mul(
                    amx[:], amx[:],
                    accx[:, bq0 * 512:(bq0 + TTILE) * 512])
                nc.vector.tensor_mul(tg[:], tg[:], amx[:])
                nc.vector.reduce_sum(
                    lg[:, sT * TTILE:(sT + 1) * TTILE]
                    .rearrange("p (c one) -> p c one", one=1),
                    tg[:].rearrange("p (c e) -> p c e", e=512),
                    axis=mybir.AxisListType.X,
                )

            for s in range(CSEG):
                cx = xpool.tile([128, CTILE * 512], bf16)
                nc.gpsimd.dma_gather(
                    out_ap=cx[:].rearrange("p (c e) -> p c e", e=512),
                    in_ap=wq,
                    idxs_ap=ctx_idx[s],
                    num_idxs=CSN, num_idxs_reg=CSN, elem_size=512,
                    queue_num=qn % 4, single_packet=False,
                )
                qn += 1
                cmf = cmpool.tile([128, CTILE * 512], bf16)
                nc.scalar.dma_start(
                    cmf[:], qmc_d[:, s * CTILE * 512:(s + 1) * CTILE * 512])
                nc.vector.tensor_mul(cx[:], cx[:], cmf[:])
                # segmented sum for b-group s: 8 shifted one-hot S matrices
                # + identity for the doc quads, accumulated in one psum
                ps = pspool.tile([128, 512], f32)
                for t in range(8):
                    nc.tensor.matmul(ps[:],
                                     lhsT=smat[:, t * 128:(t + 1) * 128],
                                     rhs=cx[:, t * 512:(t + 1) * 512],
                                     start=(t == 0), stop=False)
                nc.tensor.matmul(ps[:], lhsT=ident,
                                 rhs=docq[:, s * 512:(s + 1) * 512],
                                 start=False, stop=True)
                nc.vector.reduce_sum(
                    accf[:, s * D:(s + 1) * D]
                    .rearrange("p (one d) -> p one d", one=1),
                    ps[:].rearrange("p (q d) -> p d q", q=4, d=D),
                    axis=mybir.AxisListType.X,
                )
                for q in range(4):
                    nc.vector.tensor_copy(
                        accx[:, s * 512 + q * D:s * 512 + (q + 1) * D],
                        accf[:, s * D:(s + 1) * D])
                # phase B segs become eligible as their 4-group quadrant
                # completes (seg sT needs b-groups 4(sT%4) .. 4(sT%4)+3)
                if s % 4 == 3:
                    quad = s // 4
                    for sT in range(quad, TSEG, 4):
                        emit_tseg(sT)

            nc.sync.dma_start(vals_d[:], lg[:])

    nc.compile()
    return nc


def _get_nc():
    global _nc_cache
    if _nc_cache is None:
        _nc_cache = _build_nc()
    return _nc_cache


def _wrap16(flat):
    """[n] int array (n % 16 == 0) -> [128, n//16] int16 laid out as the
    gpsimd ucode reads it: idx j at (partition j%16, col j//16),
    replicated across the eight 16-partition groups."""
    m = np.asarray(flat, dtype=np.int16).reshape(-1, 16).T  # [16, n//16]
    return np.tile(m, (8, 1))


def _to_bf16(a):
    a = np.asarray(a, dtype=np.float32)
    if np_bf16 is not None:
        return a.astype(np_bf16)
    u = a.view(np.uint32)
    u = ((u + 0x7FFF + ((u >> 16) & 1)) >> 16).astype(np.uint16)
    return u


def _onehot_mask(sel, ntiles):
    """sel: [n] in {0..3}, slot i at gather position (i%128, i//128).
    Returns [128, ntiles*512] bf16 one-hot expanded over the 128 dims:
    m[p, ((c*4+q)*128):+128] = (sel[c*128+p]==q)."""
    n = sel.shape[0]
    assert n == ntiles * 128
    m = np.zeros((128, ntiles, 4), dtype=np.float32)
    pos = np.arange(n)
    m[pos % 128, pos // 128, sel] = 1.0
    m = np.repeat(m.reshape(128, ntiles * 4), 128, axis=1)
    return _to_bf16(m)


def _prepare_core(k, doc_ids, context_ids, sample_ids):
    """Host-side index prep for core k (pure index arithmetic)."""
    bsl = slice(k * BL, (k + 1) * BL)
    doc = np.asarray(doc_ids[bsl], dtype=np.int64)          # [BL]
    ctx = np.asarray(context_ids[bsl], dtype=np.int64)      # [BL, CTX]
    smp = np.asarray(sample_ids[bsl], dtype=np.int64)       # [BL, S]

    segs = [_wrap16(doc // 4)]
    ctxf = ctx.reshape(-1)
    for s in range(CSEG):
        segs.append(_wrap16(ctxf[s * CSN:(s + 1) * CSN] // 4))
    # T slots: slot (b, j) at gather index i = (j*16 + b//128)*128 + b%128
    b = np.repeat(np.arange(BL), S).reshape(BL, S)
    i_of = (smp * 0 + (np.arange(S)[None, :] * BQ + b // 128) * 128
            + b % 128)
    tflat = np.empty(NSMP, dtype=np.int64)
    tflat[i_of.reshape(-1)] = smp.reshape(-1)
    for s in range(TSEG):
        segs.append(_wrap16(tflat[s * TSN:(s + 1) * TSN] // 4))
    idx_all = np.concatenate(segs, axis=1)
    assert idx_all.shape == (128, IDXC), idx_all.shape

    qmd = _onehot_mask(doc % 4, BQ)
    qmc = _onehot_mask(ctxf % 4, NCTX // 128)
    qmt = _onehot_mask(tflat % 4, NSMP // 128)
    return idx_all, qmd, qmc, qmt


def _run(doc_ids, context_ids, sample_ids, paragraph_matrix, word_matrix,
         outputs, trace=False):
    _install_ntff_hook()
    from concourse.bass_utils import run_bass_kernel_spmd

    nc = _get_nc()

    wt_b = _to_bf16(np.asarray(word_matrix, dtype=np.float32))
    tt_b = _to_bf16(np.ascontiguousarray(
        np.asarray(outputs, dtype=np.float32).T))
    pt_b = _to_bf16(np.asarray(paragraph_matrix, dtype=np.float32))
    smat = np.zeros((128, 9, 128), dtype=np.float32)
    r = np.arange(128)
    for t in range(8):
        smat[r, t, r // 8 + 16 * t] = 1.0
    smat[r, 8, r] = 1.0
    smat_b = _to_bf16(smat.reshape(128, 1152))

    in_maps = []
    for k in range(N_CORES):
        idx_all, qmd, qmc, qmt = _prepare_core(k, doc_ids, context_ids,
                                               sample_ids)
        in_maps.append({
            "wt": wt_b,
            "tt": tt_b,
            "pt": pt_b,
            "idx": idx_all,
            "qmd": qmd,
            "qmc": qmc,
            "qmt": qmt,
            "smat": smat_b,
        })

    res = run_bass_kernel_spmd(nc, in_maps, core_ids=list(range(N_CORES)),
                               trace=trace)

    logits = np.zeros((B, S), dtype=np.float32)
    bl = np.arange(BL)
    for k in range(N_CORES):
        vals = np.asarray(res.results[k]["vals"],
                          dtype=np.float32).reshape(128, S, BQ)
        logits[k * BL:(k + 1) * BL] = vals[bl[:, None] % 128,
                                           np.arange(S)[None, :],
                                           bl[:, None] // 128]
    return logits, res


def kernel(doc_ids, context_ids, sample_ids, paragraph_matrix, word_matrix,
           outputs):
    logits, _ = _run(doc_ids, context_ids, sample_ids, paragraph_matrix,
                     word_matrix, outputs, trace=False)
    return logits


def kernel_traced(doc_ids, context_ids, sample_ids, paragraph_matrix,
                  word_matrix, outputs):
    """Same as kernel() but captures an NTFF profile; returns
    (logits, exec_time_ns)."""
    logits, res = _run(doc_ids, context_ids, sample_ids, paragraph_matrix,
                       word_matrix, outputs, trace=True)
    return logits, res.exec_time_ns
